# revision 20
# baseline (speedup 1.0000x reference)
"""MiniMax lightning-attention block on 8 TRN2 NeuronCores.

Sharding: token-parallel. Core c owns batch c//4, token slice (c%4)*2048.
Each core runs the blocked decay recurrence locally from a zero state, cores
AllGather their final per-head kv states (within same-batch groups of 4), and
each core applies a decayed prefix-sum of its predecessors' states as a
correction before RMSNorm / gating / output projection.

All projections and attention matmuls run bf16 x bf16 -> fp32 PSUM (full PE
rate, FWL weight loads). Spills (q*decay, o) and the output are bf16.
v is projected token-major per 4-head group (stationary = hidden tile), so
the attention inner loop needs no PE transposes for v. The RMS rstd factor is
folded into the output-projection PSUM evacuation; its reduction/broadcast
matmuls run in bf16 and are hoisted ahead of phase C so the out-proj stream
never stalls on them.
"""
from contextlib import ExitStack

import numpy as np
import ml_dtypes

import concourse.bass as bass
import concourse.tile as tile
from concourse.masks import make_identity
from concourse import bacc, mybir
from concourse.bass_utils import run_bass_kernel_spmd
F32 = mybir.dt.float32
F32R = mybir.dt.float32r
BF16 = mybir.dt.bfloat16
AF = mybir.ActivationFunctionType
ALU = mybir.AluOpType

B, N, HID, H, D = 2, 8192, 2048, 16, 128
BLOCK = 256
EPS = 1e-6
NCORES = 8
GRP = 4                 # cores per batch group
TLOC = N // GRP         # 2048 tokens per core
NBLK = TLOC // BLOCK    # 8 local blocks
KT = HID // 128         # 16 contraction tiles
CH = 2                  # phase-C chunks
CHT = TLOC // CH        # 1024 tokens per chunk
HG = 4                  # heads per v-projection group
NT = TLOC // 128        # 16 token tiles

def _build_program():
    nc = bacc.Bacc("TRN2", target_bir_lowering=False, debug=False,
                   num_devices=NCORES)

    # ---- per-core inputs (all bf16 except small fp32 tables) ----
    hT = nc.dram_tensor("hT", [HID, TLOC], BF16, kind="ExternalInput")
    wq = nc.dram_tensor("wq", [H, 128, KT * 128], BF16, kind="ExternalInput")
    wk = nc.dram_tensor("wk", [H, 128, KT * 128], BF16, kind="ExternalInput")
    wv = nc.dram_tensor("wv", [H // HG, KT, 128, HG * 128], BF16,
                        kind="ExternalInput")
    wg = nc.dram_tensor("wg", [H, 128, KT * 128], BF16, kind="ExternalInput")
    wo = nc.dram_tensor("wo", [16, 128, H * 128], BF16, kind="ExternalInput")
    htab = nc.dram_tensor("htab", [128, H * 512], BF16, kind="ExternalInput")
    qdbt = nc.dram_tensor("qdbt", [128, H * 256], BF16, kind="ExternalInput")
    ktab = nc.dram_tensor("ktab", [128, H * 11], F32, kind="ExternalInput")
    pwt = nc.dram_tensor("pwt", [128, H * GRP], F32, kind="ExternalInput")

    # ---- outputs (hid-major: final output transposed) ----
    out_t = nc.dram_tensor("out_t", [HID, TLOC], BF16, kind="ExternalOutput")

    # ---- DRAM scratch ----
    qd_sp = nc.dram_tensor("qd_sp", [H, 128, TLOC], BF16)
    o_sp = nc.dram_tensor("o_sp", [H, 128, TLOC], BF16)
    kvloc = nc.dram_tensor("kvloc", [H, 128, 128], F32)
    bar_i = nc.dram_tensor("bar_i", [1, 128], F32)
    bar_o = nc.dram_tensor("bar_o", [GRP, 128], F32)
    kvall = nc.dram_tensor("kvall", [H // 4, GRP, 4, 128, 128], F32)

    groups = [[0, 1, 2, 3], [4, 5, 6, 7]]

    with tile.TileContext(nc) as tc:
        with tc.tile_pool(name="const", bufs=1) as cpool, \
             tc.tile_pool(name="kvin", bufs=1) as kvin_pool:

            ident = cpool.tile([128, 128], BF16)
            make_identity(nc, ident[:])
            ones_col = cpool.tile([128, 1], BF16)
            nc.vector.memset(ones_col[:], 1.0)
            ones_row = cpool.tile([1, 128], BF16)
            nc.vector.memset(ones_row[:], 1.0)
            eps_t = cpool.tile([1, 1], F32)
            nc.vector.memset(eps_t[:], EPS)

            kvi = []
            with tc.tile_pool(name="hpool", bufs=1) as hpool, \
                 tc.tile_pool(name="wgpre", bufs=1) as wgpre, \
                 tc.tile_pool(name="kvex", bufs=1) as kvex:
                # ============ Phase A: per-head qkv + local attention =====
                with ExitStack() as phase_a:
                    ec = phase_a.enter_context
                    tabs = ec(tc.tile_pool(name="tabs", bufs=1))
                    wpool = ec(tc.tile_pool(name="wld", bufs=3))
                    wvpool = ec(tc.tile_pool(name="wvld", bufs=1))
                    qkv_pool = ec(tc.tile_pool(name="qkv", bufs=2))
                    vtpool = ec(tc.tile_pool(name="vtok", bufs=2))
                    opool = ec(tc.tile_pool(name="ohead", bufs=2))
                    qdpool = ec(tc.tile_pool(name="qdg", bufs=2))
                    apool = ec(tc.tile_pool(name="attn", bufs=3))
                    kvpool = ec(tc.tile_pool(name="kvstate", bufs=2))

                    # align the 4-core groups early (hides cross-core launch
                    # skew under the startup DMA window)
                    nc.gpsimd.collective_compute(
                        "AllGather", mybir.AluOpType.bypass,
                        replica_groups=groups,
                        ins=[bar_i[:]], outs=[bar_o[:]])

                    engs = (nc.scalar, nc.gpsimd, nc.sync)
                    # head 0's q/k weights lead the sync queue (first matmuls)
                    w0 = {}
                    for nm, w in (("q", wq), ("k", wk)):
                        wt = wpool.tile([128, KT * 128], BF16, tag="w")
                        nc.sync.dma_start(wt[:], w[0])
                        w0[nm] = wt
                    # hidden tiles: one contiguous 512KB transfer each
                    # (4KB bursts), round-robin over the 3 queues
                    ht = []
                    for kt in range(KT):
                        t = hpool.tile([128, TLOC], BF16, tag=f"ht{kt}")
                        engs[kt % 3].dma_start(
                            t[:], hT[kt * 128:(kt + 1) * 128, :])
                        ht.append(t)
                    # group 0's v weights + tables ride behind the hT stream
                    # (first needed ~60us in, once head 0's attention starts)
                    wv_t = wvpool.tile([128, KT * HG * 128], BF16, tag="wv")
                    for qu, eng in enumerate(
                            (nc.scalar, nc.gpsimd, nc.scalar, nc.gpsimd)):
                        ks = slice(qu * 4, (qu + 1) * 4)
                        eng.dma_start(
                            wv_t[:, qu * 2048:(qu + 1) * 2048].rearrange(
                                "p (k j) -> p k j", k=4),
                            wv[0, ks].rearrange("k p j -> p k j"))
                    ktab_t = tabs.tile([128, H * 11], F32)
                    nc.scalar.dma_start(ktab_t[:], ktab[:])
                    htab_t = tabs.tile([128, H * 512], BF16)
                    nc.gpsimd.dma_start(htab_t[:, :H * 256], htab[:, :H * 256])
                    nc.scalar.dma_start(htab_t[:, H * 256:], htab[:, H * 256:])
                    qdb_t = tabs.tile([128, H * 256], BF16)
                    nc.sync.dma_start(qdb_t[:], qdbt[:])
                    pw_all = kvex.tile([128, H * GRP], F32, tag="pwall")
                    nc.scalar.dma_start(pw_all[:], pwt[:])

                    # ---- head 0's q/k projection, kt-outer: four quarter
                    # chains accumulate in parallel across a 4-bank psum
                    # pool, so each matmul only needs its own hT tile --
                    # the PE streams while the startup DMA is still landing
                    qT0 = qkv_pool.tile([128, TLOC], BF16, tag="qT")
                    kT0 = qkv_pool.tile([128, TLOC], BF16, tag="kT")
                    with tc.tile_pool(name="pj4", bufs=4,
                                      space="PSUM") as pj4:
                        for sweep in range(2):
                            chains = []
                            for nm, dst in (("q", qT0), ("k", kT0)):
                                for qi in range(2):
                                    qu = sweep * 2 + qi
                                    ps = pj4.tile([128, 512], F32, tag="p4")
                                    chains.append((w0[nm], dst, qu, ps))
                            for kt in range(KT):
                                for wt, dst, qu, ps in chains:
                                    nc.tensor.matmul(
                                        ps[:],
                                        wt[:, kt * 128:(kt + 1) * 128],
                                        ht[kt][:, qu * 512:(qu + 1) * 512],
                                        start=(kt == 0), stop=(kt == KT - 1))
                            for wt, dst, qu, ps in chains:
                                nc.scalar.activation(
                                    dst[:, qu * 512:(qu + 1) * 512], ps[:],
                                    AF.Silu)

                    pj = ec(tc.tile_pool(name="pj", bufs=2, space="PSUM"))
                    pqk = ec(tc.tile_pool(name="pqk", bufs=2, space="PSUM"))
                    pao = ec(tc.tile_pool(name="pout", bufs=2, space="PSUM"))
                    ptp = ec(tc.tile_pool(name="ptp", bufs=1, space="PSUM"))
                    pkvp = ec(tc.tile_pool(name="pkvp", bufs=1, space="PSUM"))

                    wg_pre = []
                    gs_pre = []

                    def emit_kvi_prefix(h):
                        # decayed prefix-combine of the gathered kv states
                        # for head h; spread across A/B head slots so the
                        # DVE queue never sees a burst
                        acc = kvin_pool.tile([128, 128], BF16, tag=f"kvi{h}")
                        srcs = kvex.tile([128, GRP * 128], F32, tag="srcs")
                        nc.scalar.dma_start(
                            srcs[:].rearrange("p (j c) -> p j c", j=GRP),
                            kvall[h // 4, :, h % 4].rearrange(
                                "j p c -> p j c"))
                        accf = kvex.tile([128, 128], F32, tag="accf")
                        for j in range(GRP):
                            ssl = srcs[:, j * 128:(j + 1) * 128]
                            psc = pw_all[:, h * GRP + j:h * GRP + j + 1]
                            if j == 0:
                                nc.vector.tensor_scalar_mul(accf[:], ssl, psc)
                            elif j < GRP - 1:
                                nc.vector.scalar_tensor_tensor(
                                    accf[:], ssl, psc, accf[:],
                                    op0=ALU.mult, op1=ALU.add)
                            else:
                                nc.vector.scalar_tensor_tensor(
                                    acc[:], ssl, psc, accf[:],
                                    op0=ALU.mult, op1=ALU.add)
                        kvi.append(acc)

                    # ---- emission helpers: each returns a list of PE-dense
                    # closures (~4us each) so attention's serial kv-chain can
                    # be hidden by explicit FIFO interleaving ----
                    def make_vproj(g4, wvt):
                        vtok = vtpool.tile([128, NT * HG * 128], BF16,
                                           tag="vt")

                        def tile_unit(t):
                            def emit():
                                ps = pj.tile([128, 512], F32, tag="pj")
                                for kt in range(KT):
                                    nc.tensor.matmul(
                                        ps[:],
                                        ht[kt][:, t * 128:(t + 1) * 128],
                                        wvt[:, kt * 512:(kt + 1) * 512],
                                        start=(kt == 0), stop=(kt == KT - 1))
                                nc.scalar.activation(
                                    vtok[:, t * 512:(t + 1) * 512], ps[:],
                                    AF.Silu)
                            return emit

                        units = [tile_unit(t) for t in range(NT)]
                        if g4 < H // HG - 1:
                            nxt = wvpool.tile([128, KT * HG * 128], BF16,
                                              tag="wv")

                            def prefetch():
                                # next group's v weights on the gpsimd queue;
                                # the WAR wait drains once this group's vproj
                                # matmuls complete, long before it's needed
                                nc.gpsimd.dma_start(
                                    nxt[:].rearrange("p (k j) -> p k j",
                                                     k=KT),
                                    wv[g4 + 1].rearrange("k p j -> p k j"))
                            units.append(prefetch)
                        else:
                            nxt = None
                        return vtok, nxt, units

                    def make_qkproj(h):
                        tiles = {}
                        units = []
                        for nm, w in (("q", wq), ("k", wk)):
                            if h == 0:
                                wt = w0[nm]
                            else:
                                wt = wpool.tile([128, KT * 128], BF16, tag="w")
                                nc.sync.dma_start(wt[:], w[h])
                            dst = qkv_pool.tile([128, TLOC], BF16,
                                                tag=f"{nm}T")
                            tiles[nm] = dst

                            def qu_unit(wt, dst, qu):
                                def emit():
                                    ps = pj.tile([128, 512], F32, tag="pj")
                                    t0 = qu * 512
                                    for kt in range(KT):
                                        nc.tensor.matmul(
                                            ps[:],
                                            wt[:, kt * 128:(kt + 1) * 128],
                                            ht[kt][:, t0:t0 + 512],
                                            start=(kt == 0),
                                            stop=(kt == KT - 1))
                                    nc.scalar.activation(dst[:, t0:t0 + 512],
                                                         ps[:], AF.Silu)
                                return emit
                            units += [qu_unit(wt, dst, qu) for qu in range(4)]
                        return tiles["q"], tiles["k"], units

                    def emit_attn(h, qT, kT, vtok, fillers):
                        hh = h % HG
                        dm_t = [htab_t[:, h * 512:h * 512 + 256],
                                htab_t[:, h * 512 + 256:h * 512 + 512]]
                        kd_t = [ktab_t[:, h * 11:h * 11 + 1],
                                ktab_t[:, h * 11 + 1:h * 11 + 2]]
                        bd_t = ktab_t[:, h * 11 + 2:h * 11 + 3]
                        bdp_t = ktab_t[:, h * 11 + 3:h * 11 + 11]
                        qdb_h = qdb_t[:, h * 256:(h + 1) * 256]

                        o_head = opool.tile([128, TLOC], BF16, tag="o")
                        qdec_g = qdpool.tile([128, TLOC], BF16, tag="qd")
                        kv = kvpool.tile([128, 128], F32, tag="kv")
                        kv_bf = None
                        done = 0

                        for b in range(NBLK):
                            sl = slice(b * BLOCK, (b + 1) * BLOCK)
                            # block-local decayed q (inter decay); spill the
                            # globally-decayed version for phase C
                            if b == 0:
                                nc.vector.tensor_mul(qdec_g[:, sl], qT[:, sl],
                                                     qdb_h)
                                qdecb = qdec_g[:, sl]
                            else:
                                qdecb = apool.tile([128, BLOCK], BF16,
                                                   tag="qdec")
                                nc.vector.tensor_mul(qdecb[:], qT[:, sl],
                                                     qdb_h)
                                nc.vector.tensor_scalar_mul(
                                    qdec_g[:, sl], qdecb[:],
                                    bdp_t[:, b:b + 1])
                            # masked qk^T (m-major) + k transposes; v comes
                            # token-major from the projection.
                            # m-half1 x l-half0 is fully masked -> skip it
                            qks, vts, kts = [], [], []
                            for half in range(2):
                                mh = slice(b * BLOCK + half * 128,
                                           b * BLOCK + half * 128 + 128)
                                lw = BLOCK if half == 0 else 128
                                lsl = slice(b * BLOCK + (BLOCK - lw),
                                            (b + 1) * BLOCK)
                                pk = pqk.tile([128, BLOCK], F32, tag="pqk")
                                nc.tensor.matmul(pk[:, :lw], kT[:, mh],
                                                 qT[:, lsl],
                                                 start=True, stop=True)
                                qm = apool.tile([128, BLOCK], BF16,
                                                tag=f"qks{half}")
                                nc.vector.tensor_mul(
                                    qm[:, :lw], pk[:, :lw],
                                    dm_t[half][:, BLOCK - lw:])
                                qks.append(qm)
                                tt = 2 * b + half
                                c0 = tt * 512 + hh * 128
                                vts.append(vtok[:, c0:c0 + 128])
                                tp2 = ptp.tile([128, 128], BF16, tag="tp")
                                nc.tensor.transpose(tp2[:], kT[:, mh],
                                                    ident[:])
                                kt_ = apool.tile([128, 128], BF16,
                                                 tag=f"ktok{half}")
                                nc.vector.tensor_scalar_mul(kt_[:], tp2[:],
                                                            kd_t[half])
                                kts.append(kt_)
                            vt0, vt1 = vts[0], vts[1]
                            kt0, kt1 = kts[0][:], kts[1][:]
                            # intra (+ inter) into one psum [e, l]
                            po = pao.tile([128, BLOCK], F32, tag="po")
                            nc.tensor.matmul(po[:], vt0, qks[0][:],
                                             start=True, stop=False)
                            nc.tensor.matmul(po[:, 128:], vt1,
                                             qks[1][:, :128],
                                             start=False, stop=(b == 0),
                                             skip_group_check=True)
                            if b > 0:
                                nc.tensor.matmul(po[:], kv_bf[:], qdecb[:],
                                                 start=False, stop=True)
                            nc.vector.tensor_copy(o_head[:, sl], po[:])
                            # kv state update (fp32 state, fused decay+add)
                            pkv = pkvp.tile([128, 128], F32, tag="pkv")
                            nc.tensor.matmul(pkv[:], kt0, vt0,
                                             start=True, stop=False)
                            nc.tensor.matmul(pkv[:], kt1, vt1,
                                             start=False, stop=True)
                            if b == 0:
                                nc.vector.tensor_copy(kv[:], pkv[:])
                            else:
                                nc.vector.scalar_tensor_tensor(
                                    kv[:], kv[:], bd_t, pkv[:],
                                    op0=ALU.mult, op1=ALU.add)
                            if b < NBLK - 1:
                                kv_bf = kvpool.tile([128, 128], BF16,
                                                    tag="kvbf")
                                nc.vector.tensor_copy(kv_bf[:], kv[:])
                            # dense projection filler hides the kv chain
                            want = len(fillers) * (b + 1) // NBLK
                            while done < want:
                                fillers[done]()
                                done += 1

                        nc.sync.dma_start(o_sp[h], o_head[:])
                        nc.sync.dma_start(qd_sp[h], qdec_g[:])
                        nc.gpsimd.dma_start(kvloc[h], kv[:])
                        # gather this 4-head group's kv states early so the
                        # phase-C corrections can overlap the gate projection
                        if h % 4 == 3:
                            nc.gpsimd.collective_compute(
                                "AllGather", mybir.AluOpType.bypass,
                                replica_groups=groups,
                                ins=[kvloc[h - 3:h + 1]],
                                outs=[kvall[h // 4]])

                    def make_gate0():
                        # head 0's gate projection: filler for the last
                        # attention head, so the A->B boundary stays PE-dense.
                        # Results land in pre-B tiles consumed by phase B.
                        units = []
                        for half in range(2):
                            gsp = wgpre.tile([128, 1024], BF16,
                                             tag=f"gs0_{half}",
                                             name=f"gs0_{half}")
                            gs_pre.append(gsp)
                            for nn in range(2):
                                def emit(half=half, nn=nn, gsp=gsp):
                                    ps = pj.tile([128, 512], F32, tag="pj")
                                    t0 = half * 1024 + nn * 512
                                    for kt in range(KT):
                                        nc.tensor.matmul(
                                            ps[:],
                                            wg_pre[0][:, kt * 128:(kt + 1) * 128],
                                            ht[kt][:, t0:t0 + 512],
                                            start=(kt == 0),
                                            stop=(kt == KT - 1))
                                    nc.scalar.activation(
                                        gsp[:, nn * 512:(nn + 1) * 512],
                                        ps[:], AF.Sigmoid)
                                units.append(emit)
                        return units

                    # prologue: group 0's v projection, emitted dense
                    vtok_c, wv_t, v_units = make_vproj(0, wv_t)
                    for u in v_units:
                        u()

                    qk_next = (qT0, kT0)
                    for h in range(H):
                        qT, kT = qk_next
                        vtok_h = vtok_c
                        fillers = []
                        if h == H - 2:
                            # prefetch the first gate weights early: head 0's
                            # gate projection fills the last attention head
                            for g in range(2):
                                wt = wgpre.tile([128, KT * 128], BF16,
                                                tag=f"wgp{g}", name=f"wgp{g}")
                                nc.sync.dma_start(wt[:], wg[g])
                                wg_pre.append(wt)
                        if h == H - 1:
                            fillers += make_gate0()
                        else:
                            if h % HG == HG - 1:
                                # group tail: next group's vproj + its first
                                # head's q/k projections fill this head
                                vtok_c, wv_t, v_units = make_vproj(
                                    h // HG + 1, wv_t)
                                fillers += v_units
                            qn, kn, qk_units = make_qkproj(h + 1)
                            qk_next = (qn, kn)
                            fillers += qk_units
                        emit_attn(h, qT, kT, vtok_h, fillers)
                        if h >= 5:
                            emit_kvi_prefix(h - 5)

                # phase-C correction pools open BEFORE the gate pools so their
                # SBUF/PSUM addresses are disjoint -> corrections overlap gate
                with tc.tile_pool(name="god", bufs=2) as god_pool, \
                     tc.tile_pool(name="cstr", bufs=2) as cstr, \
                     tc.tile_pool(name="sq", bufs=1) as sqpool, \
                     tc.tile_pool(name="sac", bufs=2) as sacpool, \
                     tc.tile_pool(name="rr", bufs=1) as rpool, \
                     tc.tile_pool(name="pcorr", bufs=2, space="PSUM") as pcorr, \
                     tc.tile_pool(name="pss", bufs=1, space="PSUM") as pss:

                    gods, saccs, rbss = [], [], []
                    with tc.tile_pool(name="wgl", bufs=2) as wgpool, \
                         tc.tile_pool(name="gact", bufs=2) as gact, \
                         tc.tile_pool(name="pg", bufs=2, space="PSUM") as pg:

                        # ====== Phase B: gate projection, with phase-C
                        # corrections + gating + square-accum interleaved per
                        # head (deps are ready, so they fill no-stall slots) ==
                        for ch in range(CH):
                            god = god_pool.tile([128, H * CHT], BF16,
                                                tag="god", name=f"god{ch}")
                            sacc = sacpool.tile([128, CHT], F32R,
                                                tag="sacc", name=f"sacc{ch}")
                            gods.append(god)
                            saccs.append(sacc)
                        def emit_rstd(ch):
                            # bf16 reduce/broadcast matmuls; emitted between
                            # the last head's gate halves so the PE never
                            # idles into phase C
                            sacc = saccs[ch]
                            sab = sqpool.tile([128, CHT], BF16, tag=f"sq{ch}")
                            nc.vector.tensor_copy(sab[:], sacc[:])
                            ssp = pss.tile([1, CHT], F32, tag="ss")
                            for nn in range(2):
                                nc.tensor.matmul(ssp[:, nn * 512:(nn + 1) * 512],
                                                 ones_col[:],
                                                 sab[:, nn * 512:(nn + 1) * 512],
                                                 start=True, stop=True)
                            # rstd = exp(-0.5 ln(ms + eps)): scalar-engine
                            # only, keeps the DVE queue free at the boundary
                            r1 = rpool.tile([1, CHT], F32, tag=f"r1{ch}")
                            nc.scalar.activation(r1[:], ssp[:], AF.Ln,
                                                 bias=eps_t[:],
                                                 scale=1.0 / (H * D))
                            r3 = rpool.tile([1, CHT], F32, tag=f"r3{ch}")
                            nc.scalar.activation(r3[:], r1[:], AF.Exp,
                                                 scale=-0.5)
                            r3b = rpool.tile([1, CHT], BF16, tag=f"r3b{ch}")
                            nc.vector.tensor_copy(r3b[:], r3[:])
                            rbp = pcorr.tile([128, 512], F32, tag="pc")
                            rbp2 = pcorr.tile([128, 512], F32, tag="pc")
                            rbs = rpool.tile([128, CHT], BF16, tag=f"rbs{ch}")
                            for nn, rb in enumerate((rbp, rbp2)):
                                nc.tensor.matmul(rb[:],
                                                 ones_row[:],
                                                 r3b[:, nn * 512:(nn + 1) * 512],
                                                 start=True, stop=True)
                                nc.vector.tensor_copy(
                                    rbs[:, nn * 512:(nn + 1) * 512], rb[:])
                            rbss.append(rbs)

                        for h in range(H):
                            if h < 2:
                                wt = wg_pre[h]
                            elif True:
                                wt = wgpool.tile([128, KT * 128], BF16,
                                                 tag="wg")
                                nc.sync.dma_start(wt[:], wg[h])
                            # corrections first: their DVE chain (god add,
                            # square-accum) drains under this head's gate
                            # matmuls, so the final sacc is ready the moment
                            # the last gate matmul retires
                            gsls = []
                            for ch in range(CH):
                                csl = slice(ch * CHT, (ch + 1) * CHT)
                                god, sacc = gods[ch], saccs[ch]
                                qt = cstr.tile([128, CHT], BF16, tag="qt")
                                nc.scalar.dma_start(qt[:], qd_sp[h][:, csl])
                                ot = cstr.tile([128, CHT], BF16, tag="ot")
                                nc.gpsimd.dma_start(ot[:], o_sp[h][:, csl])
                                gsl = god[:, h * CHT:(h + 1) * CHT]
                                gsls.append(gsl)
                                for nn in range(2):
                                    nsl = slice(nn * 512, (nn + 1) * 512)
                                    pc = pcorr.tile([128, 512], F32, tag="pc")
                                    nc.tensor.matmul(
                                        pc[:], kvi[h][:], qt[:, nsl],
                                        start=True, stop=True)
                                    nc.vector.tensor_add(gsl[:, nsl], pc[:],
                                                         ot[:, nsl])
                                # squares on DVE: scalar queue stays free for
                                # the gate sigmoids
                                if h == 0:
                                    nc.vector.tensor_mul(sacc[:], gsl, gsl)
                                else:
                                    sq = sqpool.tile([128, CHT], BF16,
                                                     tag=f"sq{ch}")
                                    nc.vector.tensor_mul(sq[:], gsl, gsl)
                                    nc.vector.tensor_add(sacc[:], sacc[:],
                                                         sq[:])
                            if h == 0:
                                gss = gs_pre  # computed at the tail of A
                            else:
                                gss = []
                                for half in range(2):
                                    ps = pg.tile([128, 1024], F32, tag="pg")
                                    t0 = half * 1024
                                    for kt in range(KT):
                                        for nn in range(2):
                                            nc.tensor.matmul(
                                                ps[:, nn * 512:(nn + 1) * 512],
                                                wt[:, kt * 128:(kt + 1) * 128],
                                                ht[kt][:, t0 + nn * 512:t0 + (nn + 1) * 512],
                                                start=(kt == 0), stop=(kt == KT - 1))
                                    gs = gact.tile([128, 1024], BF16, tag="gs")
                                    nc.scalar.activation(gs[:], ps[:], AF.Sigmoid)
                                    gss.append(gs)
                            # sigmoid gate applied in place (after the
                            # square-accum reads, same DVE queue)
                            for ch in range(CH):
                                nc.vector.tensor_mul(gsls[ch], gsls[ch],
                                                     gss[ch][:])
                            if 1 <= h <= 5:
                                # tail of the prefix-combines (group 3's
                                # gather only lands after phase A ends)
                                emit_kvi_prefix(h + 10)

                    # ====== Phase C: out-proj with folded rstd ===============
                    with tc.tile_pool(name="wol", bufs=3) as wol, \
                         tc.tile_pool(name="osb", bufs=1) as osb, \
                         tc.tile_pool(name="pop", bufs=2, space="PSUM") as pop:

                        for ch in range(CH):
                            csl = slice(ch * CHT, (ch + 1) * CHT)
                            god = gods[ch]
                            # output projection (transposed): outT[j, t], x rstd
                            for jt in range(16):
                                wt = wol.tile([128, H * 128], BF16, tag="wo")
                                nc.sync.dma_start(wt[:], wo[jt])
                                po_ = pop.tile([128, CHT], F32, tag="pop")
                                for hh in range(H):
                                    for nn in range(2):
                                        nc.tensor.matmul(
                                            po_[:, nn * 512:(nn + 1) * 512],
                                            wt[:, hh * 128:(hh + 1) * 128],
                                            god[:, hh * CHT + nn * 512:hh * CHT + (nn + 1) * 512],
                                            start=(hh == 0), stop=(hh == H - 1))
                                if ch == 0 and jt == 0:
                                    # rstd rides here: its sacc deps drained
                                    # under the last gate head, its broadcast
                                    # matmuls hide under jt0's stream, and
                                    # rbs lands just before jt0's evacuation
                                    emit_rstd(0)
                                    emit_rstd(1)
                                ob = osb.tile([128, CHT], BF16, tag="ob")
                                nc.vector.tensor_mul(ob[:], po_[:], rbss[ch][:])
                                oeng = nc.sync if jt % 2 == 0 else nc.gpsimd
                                oeng.dma_start(out_t[jt * 128:(jt + 1) * 128, csl],
                                               ob[:])

    nc.compile()
    return nc


_PROGRAM = None


def _get_program():
    global _PROGRAM
    if _PROGRAM is None:
        _PROGRAM = _build_program()
    return _PROGRAM


def _host_tables(slope):
    s = slope.reshape(H, 1).astype(np.float64)
    l = np.arange(BLOCK, dtype=np.float64)
    m = np.arange(BLOCK, dtype=np.float64)
    diff = l[None, None, :] - m[None, :, None]          # [1, m, l]
    dm = np.where(diff >= 0, np.exp(-s[:, :, None] * np.maximum(diff, 0.0)), 0.0) \
        .astype(np.float32).reshape(H, 256, BLOCK)
    kd = np.exp(-s * (BLOCK - 1 - m[None, :]))          # [H, 256] f64
    qdb = np.exp(-s * (l[None, :] + 1.0))               # [H, 256] f64
    bd = np.exp(-s * BLOCK)                             # [H, 1] f64

    # htab: [128, H*512] bf16 decay masks (two m-halves per head)
    htab = np.zeros((128, H * 512), np.float32)
    for h in range(H):
        htab[:, h * 512:h * 512 + 256] = dm[h, 0:128, :]
        htab[:, h * 512 + 256:h * 512 + 512] = dm[h, 128:256, :]
    # qdbt: [128, H*256] bf16 block-local q decay broadcast over partitions
    qdbt = np.broadcast_to(
        qdb.astype(np.float32).reshape(1, H * 256), (128, H * 256)).copy()

    # ktab: [128, H*11] fp32: kd half0, kd half1, bd, bdpow[0..7] broadcast
    ktab = np.zeros((128, H * 11), np.float32)
    for h in range(H):
        ktab[:, h * 11] = kd[h, 0:128]
        ktab[:, h * 11 + 1] = kd[h, 128:256]
        ktab[:, h * 11 + 2] = bd[h, 0]
        for b in range(NBLK):
            ktab[:, h * 11 + 3 + b] = bd[h, 0] ** b
    return htab, qdbt, ktab


def _weight_tiles(w):
    # [HID, H*D] -> [H, 128, KT*128]: A[h, p, kt*128+d] = W[kt*128+p, h*128+d]
    return np.ascontiguousarray(
        w.reshape(KT, 128, H, 128).transpose(2, 1, 0, 3).reshape(H, 128, KT * 128)
    ).astype(ml_dtypes.bfloat16)


def _wv_group_tiles(w):
    # [HID, H*D] -> [H//HG, KT, 128, HG*128]:
    # A[g, kt, p, hh*128+d] = W[kt*128+p, (g*HG+hh)*128+d]
    return np.ascontiguousarray(
        w.reshape(KT, 128, H // HG, HG * 128).transpose(2, 0, 1, 3)
    ).astype(ml_dtypes.bfloat16)


def make_in_maps(inputs):
    hidden = np.asarray(inputs["hidden_states"], np.float32)
    slope = np.asarray(inputs["slope_rate"], np.float32)
    w_qkv = np.asarray(inputs["w_qkv"], np.float32)
    w_gate = np.asarray(inputs["w_gate"], np.float32)
    w_out = np.asarray(inputs["w_out"], np.float32)
    rmsw = np.asarray(inputs["rms_weight"], np.float32)

    htab, qdbt, ktab = _host_tables(slope)
    wqt = _weight_tiles(w_qkv[:, :2048])
    wkt = _weight_tiles(w_qkv[:, 2048:4096])
    wvt = _wv_group_tiles(w_qkv[:, 4096:])
    wgt = _weight_tiles(w_gate)
    wo2 = (rmsw[:, None] * w_out).astype(np.float32)
    # [HID(c), HID(j)] -> [16(jt), 128(c), H*128(jj)]
    wo = np.ascontiguousarray(
        wo2.reshape(H, 128, 16, 128).transpose(2, 1, 0, 3).reshape(16, 128, H * 128)
    ).astype(ml_dtypes.bfloat16)

    sH = slope.reshape(H).astype(np.float64)
    Dd = np.exp(-sH * TLOC)

    htab_bf = htab.astype(ml_dtypes.bfloat16)
    qdbt_bf = qdbt.astype(ml_dtypes.bfloat16)

    in_maps = []
    for c in range(NCORES):
        b, sidx = c // GRP, c % GRP
        hTc = np.ascontiguousarray(
            hidden[b, sidx * TLOC:(sidx + 1) * TLOC, :].T
        ).astype(ml_dtypes.bfloat16)
        pw = np.zeros((H, GRP), np.float64)
        for j in range(sidx):
            pw[:, j] = Dd ** (sidx - 1 - j)
        pwt = np.broadcast_to(pw.astype(np.float32).reshape(1, H * GRP),
                              (128, H * GRP)).copy()
        in_maps.append(dict(hT=hTc, wq=wqt, wk=wkt, wv=wvt, wg=wgt, wo=wo,
                            htab=htab_bf, qdbt=qdbt_bf, ktab=ktab, pwt=pwt))
    return in_maps


def assemble_out(results):
    out = np.zeros((B, N, HID), np.float32)
    for c in range(NCORES):
        b, sidx = c // GRP, c % GRP
        out[b, sidx * TLOC:(sidx + 1) * TLOC, :] = \
            results[c]["out_t"].astype(np.float32).T
    return out


def kernel(**inputs):
    in_maps = make_in_maps(inputs)
    nc = _get_program()
    res = run_bass_kernel_spmd(nc, in_maps, core_ids=list(range(NCORES)))
    return assemble_out(res.results)


# revision 21
# speedup vs baseline: 1.0471x; 1.0471x over previous
"""MiniMax lightning-attention block on 8 TRN2 NeuronCores.

Sharding: token-parallel. Core c owns batch c//4, token slice (c%4)*2048.
Each core runs the blocked decay recurrence locally from a zero state, cores
AllGather their final per-head kv states (within same-batch groups of 4), and
each core applies a decayed prefix-sum of its predecessors' states as a
correction before RMSNorm / gating / output projection.

All projections and attention matmuls run bf16 x bf16 -> fp32 PSUM (full PE
rate, FWL weight loads). Spills (q*decay, o) and the output are bf16.
v is projected token-major per 4-head group (stationary = hidden tile), so
the attention inner loop needs no PE transposes for v. The RMS rstd factor is
folded into the output-projection PSUM evacuation; its reduction/broadcast
matmuls run in bf16 and are hoisted ahead of phase C so the out-proj stream
never stalls on them.
"""
from contextlib import ExitStack

import numpy as np
import ml_dtypes

import concourse.bass as bass
import concourse.tile as tile
from concourse.masks import make_identity
from concourse import bacc, mybir
from concourse.bass_utils import run_bass_kernel_spmd
F32 = mybir.dt.float32
F32R = mybir.dt.float32r
BF16 = mybir.dt.bfloat16
AF = mybir.ActivationFunctionType
ALU = mybir.AluOpType

B, N, HID, H, D = 2, 8192, 2048, 16, 128
BLOCK = 256
EPS = 1e-6
NCORES = 8
GRP = 4                 # cores per batch group
TLOC = N // GRP         # 2048 tokens per core
NBLK = TLOC // BLOCK    # 8 local blocks
KT = HID // 128         # 16 contraction tiles
CH = 2                  # phase-C chunks
CHT = TLOC // CH        # 1024 tokens per chunk
HG = 4                  # heads per v-projection group
NT = TLOC // 128        # 16 token tiles

def _build_program():
    nc = bacc.Bacc("TRN2", target_bir_lowering=False, debug=False,
                   num_devices=NCORES)

    # ---- per-core inputs (all bf16 except small fp32 tables) ----
    hT = nc.dram_tensor("hT", [HID, TLOC], BF16, kind="ExternalInput")
    wq = nc.dram_tensor("wq", [H, 128, KT * 128], BF16, kind="ExternalInput")
    wk = nc.dram_tensor("wk", [H, 128, KT * 128], BF16, kind="ExternalInput")
    wv = nc.dram_tensor("wv", [H // HG, KT, 128, HG * 128], BF16,
                        kind="ExternalInput")
    wg = nc.dram_tensor("wg", [H, 128, KT * 128], BF16, kind="ExternalInput")
    wo = nc.dram_tensor("wo", [16, 128, H * 128], BF16, kind="ExternalInput")
    htab = nc.dram_tensor("htab", [128, H * 512], BF16, kind="ExternalInput")
    qdbt = nc.dram_tensor("qdbt", [128, H * 256], BF16, kind="ExternalInput")
    ktab = nc.dram_tensor("ktab", [128, H * 11], F32, kind="ExternalInput")
    pwt = nc.dram_tensor("pwt", [128, H * GRP], F32, kind="ExternalInput")

    # ---- outputs (hid-major: final output transposed) ----
    out_t = nc.dram_tensor("out_t", [HID, TLOC], BF16, kind="ExternalOutput")

    # ---- DRAM scratch ----
    qd_sp = nc.dram_tensor("qd_sp", [H, 128, TLOC], BF16)
    o_sp = nc.dram_tensor("o_sp", [H, 128, TLOC], BF16)
    kvloc = nc.dram_tensor("kvloc", [H, 128, 128], F32)
    bar_i = nc.dram_tensor("bar_i", [1, 128], F32)
    bar_o = nc.dram_tensor("bar_o", [GRP, 128], F32)
    kvall = nc.dram_tensor("kvall", [H // 4, GRP, 4, 128, 128], F32)

    groups = [[0, 1, 2, 3], [4, 5, 6, 7]]

    with tile.TileContext(nc) as tc:
        with tc.tile_pool(name="const", bufs=1) as cpool, \
             tc.tile_pool(name="kvin", bufs=1) as kvin_pool:

            ident = cpool.tile([128, 128], BF16)
            make_identity(nc, ident[:])
            ones_col = cpool.tile([128, 1], BF16)
            nc.vector.memset(ones_col[:], 1.0)
            ones_row = cpool.tile([1, 128], BF16)
            nc.vector.memset(ones_row[:], 1.0)
            eps_t = cpool.tile([1, 1], F32)
            nc.vector.memset(eps_t[:], EPS)

            kvi = []
            with tc.tile_pool(name="hpool", bufs=1) as hpool, \
                 tc.tile_pool(name="wgpre", bufs=1) as wgpre, \
                 tc.tile_pool(name="kvex", bufs=1) as kvex:
                # ============ Phase A: per-head qkv + local attention =====
                with ExitStack() as phase_a:
                    ec = phase_a.enter_context
                    tabs = ec(tc.tile_pool(name="tabs", bufs=1))
                    wpool = ec(tc.tile_pool(name="wld", bufs=3))
                    wvpool = ec(tc.tile_pool(name="wvld", bufs=1))
                    qkv_pool = ec(tc.tile_pool(name="qkv", bufs=2))
                    vtpool = ec(tc.tile_pool(name="vtok", bufs=2))
                    opool = ec(tc.tile_pool(name="ohead", bufs=2))
                    qdpool = ec(tc.tile_pool(name="qdg", bufs=2))
                    apool = ec(tc.tile_pool(name="attn", bufs=3))
                    kvpool = ec(tc.tile_pool(name="kvstate", bufs=2))

                    # align the 4-core groups early (hides cross-core launch
                    # skew under the startup DMA window)
                    nc.gpsimd.collective_compute(
                        "AllGather", mybir.AluOpType.bypass,
                        replica_groups=groups,
                        ins=[bar_i[:]], outs=[bar_o[:]])

                    engs = (nc.scalar, nc.gpsimd, nc.sync)
                    # head 0's q/k weights lead the sync queue (first matmuls)
                    w0 = {}
                    for nm, w in (("q", wq), ("k", wk)):
                        wt = wpool.tile([128, KT * 128], BF16, tag="w")
                        nc.sync.dma_start(wt[:], w[0])
                        w0[nm] = wt
                    # hidden tiles: one contiguous 512KB transfer each
                    # (4KB bursts), round-robin over the 3 queues
                    ht = []
                    for kt in range(KT):
                        t = hpool.tile([128, TLOC], BF16, tag=f"ht{kt}")
                        engs[kt % 3].dma_start(
                            t[:], hT[kt * 128:(kt + 1) * 128, :])
                        ht.append(t)
                    # group 0's v weights behind the hT shares (needed once
                    # head 0's q/k sweeps finish, ~45us in)
                    wv_t = wvpool.tile([128, KT * HG * 128], BF16, tag="wv")
                    for qu, eng in enumerate((nc.scalar, nc.gpsimd)):
                        ks = slice(qu * 8, (qu + 1) * 8)
                        eng.dma_start(
                            wv_t[:, qu * 4096:(qu + 1) * 4096].rearrange(
                                "p (k j) -> p k j", k=8),
                            wv[0, ks].rearrange("k p j -> p k j"))
                    ktab_t = tabs.tile([128, H * 11], F32)
                    nc.scalar.dma_start(ktab_t[:], ktab[:])
                    htab_t = tabs.tile([128, H * 512], BF16)
                    nc.gpsimd.dma_start(htab_t[:, :H * 256], htab[:, :H * 256])
                    nc.scalar.dma_start(htab_t[:, H * 256:], htab[:, H * 256:])
                    qdb_t = tabs.tile([128, H * 256], BF16)
                    nc.sync.dma_start(qdb_t[:], qdbt[:])
                    pw_all = kvex.tile([128, H * GRP], F32, tag="pwall")
                    nc.scalar.dma_start(pw_all[:], pwt[:])

                    # ---- head 0's q/k projection, kt-outer: four quarter
                    # chains accumulate in parallel across a 4-bank psum
                    # pool, so each matmul only needs its own hT tile --
                    # the PE streams while the startup DMA is still landing
                    qT0 = qkv_pool.tile([128, TLOC], BF16, tag="qT")
                    kT0 = qkv_pool.tile([128, TLOC], BF16, tag="kT")
                    with tc.tile_pool(name="pj4", bufs=4,
                                      space="PSUM") as pj4:
                        for sweep in range(2):
                            chains = []
                            for nm, dst in (("q", qT0), ("k", kT0)):
                                for qi in range(2):
                                    qu = sweep * 2 + qi
                                    ps = pj4.tile([128, 512], F32, tag="p4")
                                    chains.append((w0[nm], dst, qu, ps))
                            for kt in range(KT):
                                for wt, dst, qu, ps in chains:
                                    nc.tensor.matmul(
                                        ps[:],
                                        wt[:, kt * 128:(kt + 1) * 128],
                                        ht[kt][:, qu * 512:(qu + 1) * 512],
                                        start=(kt == 0), stop=(kt == KT - 1))
                            for wt, dst, qu, ps in chains:
                                nc.scalar.activation(
                                    dst[:, qu * 512:(qu + 1) * 512], ps[:],
                                    AF.Silu)

                    pj = ec(tc.tile_pool(name="pj", bufs=2, space="PSUM"))
                    pqk = ec(tc.tile_pool(name="pqk", bufs=2, space="PSUM"))
                    pao = ec(tc.tile_pool(name="pout", bufs=2, space="PSUM"))
                    ptp = ec(tc.tile_pool(name="ptp", bufs=1, space="PSUM"))
                    pkvp = ec(tc.tile_pool(name="pkvp", bufs=1, space="PSUM"))

                    wg_pre = []
                    gs_pre = []

                    def emit_kvi_prefix(h):
                        # decayed prefix-combine of the gathered kv states
                        # for head h; spread across A/B head slots so the
                        # DVE queue never sees a burst
                        acc = kvin_pool.tile([128, 128], BF16, tag=f"kvi{h}")
                        srcs = kvex.tile([128, GRP * 128], F32, tag="srcs")
                        nc.scalar.dma_start(
                            srcs[:].rearrange("p (j c) -> p j c", j=GRP),
                            kvall[h // 4, :, h % 4].rearrange(
                                "j p c -> p j c"))
                        accf = kvex.tile([128, 128], F32, tag="accf")
                        for j in range(GRP):
                            ssl = srcs[:, j * 128:(j + 1) * 128]
                            psc = pw_all[:, h * GRP + j:h * GRP + j + 1]
                            if j == 0:
                                nc.vector.tensor_scalar_mul(accf[:], ssl, psc)
                            elif j < GRP - 1:
                                nc.vector.scalar_tensor_tensor(
                                    accf[:], ssl, psc, accf[:],
                                    op0=ALU.mult, op1=ALU.add)
                            else:
                                nc.vector.scalar_tensor_tensor(
                                    acc[:], ssl, psc, accf[:],
                                    op0=ALU.mult, op1=ALU.add)
                        kvi.append(acc)

                    # ---- emission helpers: each returns a list of PE-dense
                    # closures (~4us each) so attention's serial kv-chain can
                    # be hidden by explicit FIFO interleaving ----
                    def make_vproj(g4, wvt):
                        vtok = vtpool.tile([128, NT * HG * 128], BF16,
                                           tag="vt")

                        def tile_unit(t):
                            def emit():
                                ps = pj.tile([128, 512], F32, tag="pj")
                                for kt in range(KT):
                                    nc.tensor.matmul(
                                        ps[:],
                                        ht[kt][:, t * 128:(t + 1) * 128],
                                        wvt[:, kt * 512:(kt + 1) * 512],
                                        start=(kt == 0), stop=(kt == KT - 1))
                                nc.scalar.activation(
                                    vtok[:, t * 512:(t + 1) * 512], ps[:],
                                    AF.Silu)
                            return emit

                        units = [tile_unit(t) for t in range(NT)]
                        if g4 < H // HG - 1:
                            nxt = wvpool.tile([128, KT * HG * 128], BF16,
                                              tag="wv")

                            def prefetch():
                                # next group's v weights on the gpsimd queue;
                                # the WAR wait drains once this group's vproj
                                # matmuls complete, long before it's needed
                                nc.gpsimd.dma_start(
                                    nxt[:].rearrange("p (k j) -> p k j",
                                                     k=KT),
                                    wv[g4 + 1].rearrange("k p j -> p k j"))
                            units.append(prefetch)
                        else:
                            nxt = None
                        return vtok, nxt, units

                    def make_qkproj(h):
                        tiles = {}
                        units = []
                        for nm, w in (("q", wq), ("k", wk)):
                            if h == 0:
                                wt = w0[nm]
                            else:
                                wt = wpool.tile([128, KT * 128], BF16, tag="w")
                                nc.sync.dma_start(wt[:], w[h])
                            dst = qkv_pool.tile([128, TLOC], BF16,
                                                tag=f"{nm}T")
                            tiles[nm] = dst

                            def qu_unit(wt, dst, qu):
                                def emit():
                                    ps = pj.tile([128, 512], F32, tag="pj")
                                    t0 = qu * 512
                                    for kt in range(KT):
                                        nc.tensor.matmul(
                                            ps[:],
                                            wt[:, kt * 128:(kt + 1) * 128],
                                            ht[kt][:, t0:t0 + 512],
                                            start=(kt == 0),
                                            stop=(kt == KT - 1))
                                    nc.scalar.activation(dst[:, t0:t0 + 512],
                                                         ps[:], AF.Silu)
                                return emit
                            units += [qu_unit(wt, dst, qu) for qu in range(4)]
                        return tiles["q"], tiles["k"], units

                    def emit_attn(h, qT, kT, vtok, fillers):
                        hh = h % HG
                        dm_t = [htab_t[:, h * 512:h * 512 + 256],
                                htab_t[:, h * 512 + 256:h * 512 + 512]]
                        kd_t = [ktab_t[:, h * 11:h * 11 + 1],
                                ktab_t[:, h * 11 + 1:h * 11 + 2]]
                        bd_t = ktab_t[:, h * 11 + 2:h * 11 + 3]
                        bdp_t = ktab_t[:, h * 11 + 3:h * 11 + 11]
                        qdb_h = qdb_t[:, h * 256:(h + 1) * 256]

                        o_head = opool.tile([128, TLOC], BF16, tag="o")
                        qdec_g = qdpool.tile([128, TLOC], BF16, tag="qd")
                        kv = kvpool.tile([128, 128], F32, tag="kv")
                        kv_bf = None
                        done = 0

                        for b in range(NBLK):
                            sl = slice(b * BLOCK, (b + 1) * BLOCK)
                            # block-local decayed q (inter decay); spill the
                            # globally-decayed version for phase C
                            if b == 0:
                                nc.vector.tensor_mul(qdec_g[:, sl], qT[:, sl],
                                                     qdb_h)
                                qdecb = qdec_g[:, sl]
                            else:
                                qdecb = apool.tile([128, BLOCK], BF16,
                                                   tag="qdec")
                                nc.vector.tensor_mul(qdecb[:], qT[:, sl],
                                                     qdb_h)
                                nc.vector.tensor_scalar_mul(
                                    qdec_g[:, sl], qdecb[:],
                                    bdp_t[:, b:b + 1])
                            # masked qk^T (m-major) + k transposes; v comes
                            # token-major from the projection.
                            # m-half1 x l-half0 is fully masked -> skip it
                            qks, vts, kts = [], [], []
                            for half in range(2):
                                mh = slice(b * BLOCK + half * 128,
                                           b * BLOCK + half * 128 + 128)
                                lw = BLOCK if half == 0 else 128
                                lsl = slice(b * BLOCK + (BLOCK - lw),
                                            (b + 1) * BLOCK)
                                pk = pqk.tile([128, BLOCK], F32, tag="pqk")
                                nc.tensor.matmul(pk[:, :lw], kT[:, mh],
                                                 qT[:, lsl],
                                                 start=True, stop=True)
                                qm = apool.tile([128, BLOCK], BF16,
                                                tag=f"qks{half}")
                                nc.vector.tensor_mul(
                                    qm[:, :lw], pk[:, :lw],
                                    dm_t[half][:, BLOCK - lw:])
                                qks.append(qm)
                                tt = 2 * b + half
                                c0 = tt * 512 + hh * 128
                                vts.append(vtok[:, c0:c0 + 128])
                                tp2 = ptp.tile([128, 128], BF16, tag="tp")
                                nc.tensor.transpose(tp2[:], kT[:, mh],
                                                    ident[:])
                                kt_ = apool.tile([128, 128], BF16,
                                                 tag=f"ktok{half}")
                                nc.vector.tensor_scalar_mul(kt_[:], tp2[:],
                                                            kd_t[half])
                                kts.append(kt_)
                            vt0, vt1 = vts[0], vts[1]
                            kt0, kt1 = kts[0][:], kts[1][:]
                            # intra (+ inter) into one psum [e, l]
                            po = pao.tile([128, BLOCK], F32, tag="po")
                            nc.tensor.matmul(po[:], vt0, qks[0][:],
                                             start=True, stop=False)
                            nc.tensor.matmul(po[:, 128:], vt1,
                                             qks[1][:, :128],
                                             start=False, stop=(b == 0),
                                             skip_group_check=True)
                            if b > 0:
                                nc.tensor.matmul(po[:], kv_bf[:], qdecb[:],
                                                 start=False, stop=True)
                            nc.vector.tensor_copy(o_head[:, sl], po[:])
                            # kv state update (fp32 state, fused decay+add)
                            pkv = pkvp.tile([128, 128], F32, tag="pkv")
                            nc.tensor.matmul(pkv[:], kt0, vt0,
                                             start=True, stop=False)
                            nc.tensor.matmul(pkv[:], kt1, vt1,
                                             start=False, stop=True)
                            if b == 0:
                                nc.vector.tensor_copy(kv[:], pkv[:])
                            else:
                                nc.vector.scalar_tensor_tensor(
                                    kv[:], kv[:], bd_t, pkv[:],
                                    op0=ALU.mult, op1=ALU.add)
                            if b < NBLK - 1:
                                kv_bf = kvpool.tile([128, 128], BF16,
                                                    tag="kvbf")
                                nc.vector.tensor_copy(kv_bf[:], kv[:])
                            # dense projection filler hides the kv chain
                            want = len(fillers) * (b + 1) // NBLK
                            while done < want:
                                fillers[done]()
                                done += 1

                        nc.sync.dma_start(o_sp[h], o_head[:])
                        nc.sync.dma_start(qd_sp[h], qdec_g[:])
                        nc.gpsimd.dma_start(kvloc[h], kv[:])
                        # gather this 4-head group's kv states early so the
                        # phase-C corrections can overlap the gate projection
                        if h % 4 == 3:
                            nc.gpsimd.collective_compute(
                                "AllGather", mybir.AluOpType.bypass,
                                replica_groups=groups,
                                ins=[kvloc[h - 3:h + 1]],
                                outs=[kvall[h // 4]])

                    def make_gate0():
                        # head 0's gate projection: filler for the last
                        # attention head, so the A->B boundary stays PE-dense.
                        # Results land in pre-B tiles consumed by phase B.
                        units = []
                        for half in range(2):
                            gsp = wgpre.tile([128, 1024], BF16,
                                             tag=f"gs0_{half}",
                                             name=f"gs0_{half}")
                            gs_pre.append(gsp)
                            for nn in range(2):
                                def emit(half=half, nn=nn, gsp=gsp):
                                    ps = pj.tile([128, 512], F32, tag="pj")
                                    t0 = half * 1024 + nn * 512
                                    for kt in range(KT):
                                        nc.tensor.matmul(
                                            ps[:],
                                            wg_pre[0][:, kt * 128:(kt + 1) * 128],
                                            ht[kt][:, t0:t0 + 512],
                                            start=(kt == 0),
                                            stop=(kt == KT - 1))
                                    nc.scalar.activation(
                                        gsp[:, nn * 512:(nn + 1) * 512],
                                        ps[:], AF.Sigmoid)
                                units.append(emit)
                        return units

                    # prologue: group 0's v projection, emitted dense
                    vtok_c, wv_t, v_units = make_vproj(0, wv_t)
                    for u in v_units:
                        u()

                    qk_next = (qT0, kT0)
                    for h in range(H):
                        qT, kT = qk_next
                        vtok_h = vtok_c
                        fillers = []
                        if h == H - 2:
                            # prefetch the first gate weights early: head 0's
                            # gate projection fills the last attention head
                            for g in range(2):
                                wt = wgpre.tile([128, KT * 128], BF16,
                                                tag=f"wgp{g}", name=f"wgp{g}")
                                nc.sync.dma_start(wt[:], wg[g])
                                wg_pre.append(wt)
                        if h == H - 1:
                            fillers += make_gate0()
                        else:
                            if h % HG == HG - 2 and h < H - 2:
                                # next group's vproj fills the second-to-last
                                # head of this group, so its weight prefetch
                                # (WAR on this group's vproj) drains before
                                # the group-tail AllGather posts on gpsimd
                                v_next = make_vproj(h // HG + 1, wv_t)
                                vtok_n, wv_t, v_units = v_next
                                fillers += v_units
                            qn, kn, qk_units = make_qkproj(h + 1)
                            qk_next = (qn, kn)
                            fillers += qk_units
                        emit_attn(h, qT, kT, vtok_h, fillers)
                        if h % HG == HG - 1:
                            vtok_c = vtok_n
                        if h >= 6:
                            emit_kvi_prefix(h - 6)

                # phase-C correction pools open BEFORE the gate pools so their
                # SBUF/PSUM addresses are disjoint -> corrections overlap gate
                with tc.tile_pool(name="god", bufs=2) as god_pool, \
                     tc.tile_pool(name="cstr", bufs=2) as cstr, \
                     tc.tile_pool(name="sq", bufs=1) as sqpool, \
                     tc.tile_pool(name="sac", bufs=2) as sacpool, \
                     tc.tile_pool(name="rr", bufs=1) as rpool, \
                     tc.tile_pool(name="pcorr", bufs=2, space="PSUM") as pcorr, \
                     tc.tile_pool(name="pss", bufs=1, space="PSUM") as pss:

                    gods, saccs, rbss = [], [], []
                    with tc.tile_pool(name="wgl", bufs=2) as wgpool, \
                         tc.tile_pool(name="gact", bufs=2) as gact, \
                         tc.tile_pool(name="pg", bufs=2, space="PSUM") as pg:

                        # ====== Phase B: gate projection, with phase-C
                        # corrections + gating + square-accum interleaved per
                        # head (deps are ready, so they fill no-stall slots) ==
                        for ch in range(CH):
                            god = god_pool.tile([128, H * CHT], BF16,
                                                tag="god", name=f"god{ch}")
                            sacc = sacpool.tile([128, CHT], F32R,
                                                tag="sacc", name=f"sacc{ch}")
                            gods.append(god)
                            saccs.append(sacc)
                        def emit_rstd(ch):
                            # bf16 reduce/broadcast matmuls; emitted between
                            # the last head's gate halves so the PE never
                            # idles into phase C
                            sacc = saccs[ch]
                            sab = sqpool.tile([128, CHT], BF16, tag=f"sq{ch}")
                            nc.vector.tensor_copy(sab[:], sacc[:])
                            ssp = pss.tile([1, CHT], F32, tag="ss")
                            for nn in range(2):
                                nc.tensor.matmul(ssp[:, nn * 512:(nn + 1) * 512],
                                                 ones_col[:],
                                                 sab[:, nn * 512:(nn + 1) * 512],
                                                 start=True, stop=True)
                            # rstd = exp(-0.5 ln(ms + eps)): scalar-engine
                            # only, keeps the DVE queue free at the boundary
                            r1 = rpool.tile([1, CHT], F32, tag=f"r1{ch}")
                            nc.scalar.activation(r1[:], ssp[:], AF.Ln,
                                                 bias=eps_t[:],
                                                 scale=1.0 / (H * D))
                            r3 = rpool.tile([1, CHT], F32, tag=f"r3{ch}")
                            nc.scalar.activation(r3[:], r1[:], AF.Exp,
                                                 scale=-0.5)
                            r3b = rpool.tile([1, CHT], BF16, tag=f"r3b{ch}")
                            nc.vector.tensor_copy(r3b[:], r3[:])
                            rbp = pcorr.tile([128, 512], F32, tag="pc")
                            rbp2 = pcorr.tile([128, 512], F32, tag="pc")
                            rbs = rpool.tile([128, CHT], BF16, tag=f"rbs{ch}")
                            for nn, rb in enumerate((rbp, rbp2)):
                                nc.tensor.matmul(rb[:],
                                                 ones_row[:],
                                                 r3b[:, nn * 512:(nn + 1) * 512],
                                                 start=True, stop=True)
                                nc.vector.tensor_copy(
                                    rbs[:, nn * 512:(nn + 1) * 512], rb[:])
                            rbss.append(rbs)

                        for h in range(H):
                            if h < 2:
                                wt = wg_pre[h]
                            elif True:
                                wt = wgpool.tile([128, KT * 128], BF16,
                                                 tag="wg")
                                nc.sync.dma_start(wt[:], wg[h])
                            # corrections first: their DVE chain (god add,
                            # square-accum) drains under this head's gate
                            # matmuls, so the final sacc is ready the moment
                            # the last gate matmul retires
                            gsls = []
                            for ch in range(CH):
                                csl = slice(ch * CHT, (ch + 1) * CHT)
                                god, sacc = gods[ch], saccs[ch]
                                qt = cstr.tile([128, CHT], BF16, tag="qt")
                                nc.scalar.dma_start(qt[:], qd_sp[h][:, csl])
                                ot = cstr.tile([128, CHT], BF16, tag="ot")
                                nc.gpsimd.dma_start(ot[:], o_sp[h][:, csl])
                                gsl = god[:, h * CHT:(h + 1) * CHT]
                                gsls.append(gsl)
                                for nn in range(2):
                                    nsl = slice(nn * 512, (nn + 1) * 512)
                                    pc = pcorr.tile([128, 512], F32, tag="pc")
                                    nc.tensor.matmul(
                                        pc[:], kvi[h][:], qt[:, nsl],
                                        start=True, stop=True)
                                    nc.vector.tensor_add(gsl[:, nsl], pc[:],
                                                         ot[:, nsl])
                                # squares on DVE: scalar queue stays free for
                                # the gate sigmoids
                                if h == 0:
                                    nc.vector.tensor_mul(sacc[:], gsl, gsl)
                                else:
                                    sq = sqpool.tile([128, CHT], BF16,
                                                     tag=f"sq{ch}")
                                    nc.vector.tensor_mul(sq[:], gsl, gsl)
                                    nc.vector.tensor_add(sacc[:], sacc[:],
                                                         sq[:])
                            if h == 0:
                                gss = gs_pre  # computed at the tail of A
                            else:
                                gss = []
                                for half in range(2):
                                    ps = pg.tile([128, 1024], F32, tag="pg")
                                    t0 = half * 1024
                                    for kt in range(KT):
                                        for nn in range(2):
                                            nc.tensor.matmul(
                                                ps[:, nn * 512:(nn + 1) * 512],
                                                wt[:, kt * 128:(kt + 1) * 128],
                                                ht[kt][:, t0 + nn * 512:t0 + (nn + 1) * 512],
                                                start=(kt == 0), stop=(kt == KT - 1))
                                    gs = gact.tile([128, 1024], BF16, tag="gs")
                                    nc.scalar.activation(gs[:], ps[:], AF.Sigmoid)
                                    gss.append(gs)
                            # sigmoid gate applied in place (after the
                            # square-accum reads, same DVE queue)
                            for ch in range(CH):
                                nc.vector.tensor_mul(gsls[ch], gsls[ch],
                                                     gss[ch][:])
                            if h <= 5:
                                # tail of the prefix-combines (group 3's
                                # gather only lands after phase A ends)
                                emit_kvi_prefix(h + 10)

                    # ====== Phase C: out-proj with folded rstd ===============
                    with tc.tile_pool(name="wol", bufs=3) as wol, \
                         tc.tile_pool(name="osb", bufs=1) as osb, \
                         tc.tile_pool(name="pop", bufs=2, space="PSUM") as pop:

                        for ch in range(CH):
                            csl = slice(ch * CHT, (ch + 1) * CHT)
                            god = gods[ch]
                            # output projection (transposed): outT[j, t], x rstd
                            for jt in range(16):
                                wt = wol.tile([128, H * 128], BF16, tag="wo")
                                nc.sync.dma_start(wt[:], wo[jt])
                                po_ = pop.tile([128, CHT], F32, tag="pop")
                                for hh in range(H):
                                    for nn in range(2):
                                        nc.tensor.matmul(
                                            po_[:, nn * 512:(nn + 1) * 512],
                                            wt[:, hh * 128:(hh + 1) * 128],
                                            god[:, hh * CHT + nn * 512:hh * CHT + (nn + 1) * 512],
                                            start=(hh == 0), stop=(hh == H - 1))
                                if ch == 0 and jt == 0:
                                    # rstd rides here: its sacc deps drained
                                    # under the last gate head, its broadcast
                                    # matmuls hide under jt0's stream, and
                                    # rbs lands just before jt0's evacuation
                                    emit_rstd(0)
                                    emit_rstd(1)
                                ob = osb.tile([128, CHT], BF16, tag="ob")
                                nc.vector.tensor_mul(ob[:], po_[:], rbss[ch][:])
                                oeng = nc.sync if jt % 2 == 0 else nc.gpsimd
                                oeng.dma_start(out_t[jt * 128:(jt + 1) * 128, csl],
                                               ob[:])

    nc.compile()
    return nc


_PROGRAM = None


def _get_program():
    global _PROGRAM
    if _PROGRAM is None:
        _PROGRAM = _build_program()
    return _PROGRAM


def _host_tables(slope):
    s = slope.reshape(H, 1).astype(np.float64)
    l = np.arange(BLOCK, dtype=np.float64)
    m = np.arange(BLOCK, dtype=np.float64)
    diff = l[None, None, :] - m[None, :, None]          # [1, m, l]
    dm = np.where(diff >= 0, np.exp(-s[:, :, None] * np.maximum(diff, 0.0)), 0.0) \
        .astype(np.float32).reshape(H, 256, BLOCK)
    kd = np.exp(-s * (BLOCK - 1 - m[None, :]))          # [H, 256] f64
    qdb = np.exp(-s * (l[None, :] + 1.0))               # [H, 256] f64
    bd = np.exp(-s * BLOCK)                             # [H, 1] f64

    # htab: [128, H*512] bf16 decay masks (two m-halves per head)
    htab = np.zeros((128, H * 512), np.float32)
    for h in range(H):
        htab[:, h * 512:h * 512 + 256] = dm[h, 0:128, :]
        htab[:, h * 512 + 256:h * 512 + 512] = dm[h, 128:256, :]
    # qdbt: [128, H*256] bf16 block-local q decay broadcast over partitions
    qdbt = np.broadcast_to(
        qdb.astype(np.float32).reshape(1, H * 256), (128, H * 256)).copy()

    # ktab: [128, H*11] fp32: kd half0, kd half1, bd, bdpow[0..7] broadcast
    ktab = np.zeros((128, H * 11), np.float32)
    for h in range(H):
        ktab[:, h * 11] = kd[h, 0:128]
        ktab[:, h * 11 + 1] = kd[h, 128:256]
        ktab[:, h * 11 + 2] = bd[h, 0]
        for b in range(NBLK):
            ktab[:, h * 11 + 3 + b] = bd[h, 0] ** b
    return htab, qdbt, ktab


def _weight_tiles(w):
    # [HID, H*D] -> [H, 128, KT*128]: A[h, p, kt*128+d] = W[kt*128+p, h*128+d]
    return np.ascontiguousarray(
        w.reshape(KT, 128, H, 128).transpose(2, 1, 0, 3).reshape(H, 128, KT * 128)
    ).astype(ml_dtypes.bfloat16)


def _wv_group_tiles(w):
    # [HID, H*D] -> [H//HG, KT, 128, HG*128]:
    # A[g, kt, p, hh*128+d] = W[kt*128+p, (g*HG+hh)*128+d]
    return np.ascontiguousarray(
        w.reshape(KT, 128, H // HG, HG * 128).transpose(2, 0, 1, 3)
    ).astype(ml_dtypes.bfloat16)


def make_in_maps(inputs):
    hidden = np.asarray(inputs["hidden_states"], np.float32)
    slope = np.asarray(inputs["slope_rate"], np.float32)
    w_qkv = np.asarray(inputs["w_qkv"], np.float32)
    w_gate = np.asarray(inputs["w_gate"], np.float32)
    w_out = np.asarray(inputs["w_out"], np.float32)
    rmsw = np.asarray(inputs["rms_weight"], np.float32)

    htab, qdbt, ktab = _host_tables(slope)
    wqt = _weight_tiles(w_qkv[:, :2048])
    wkt = _weight_tiles(w_qkv[:, 2048:4096])
    wvt = _wv_group_tiles(w_qkv[:, 4096:])
    wgt = _weight_tiles(w_gate)
    wo2 = (rmsw[:, None] * w_out).astype(np.float32)
    # [HID(c), HID(j)] -> [16(jt), 128(c), H*128(jj)]
    wo = np.ascontiguousarray(
        wo2.reshape(H, 128, 16, 128).transpose(2, 1, 0, 3).reshape(16, 128, H * 128)
    ).astype(ml_dtypes.bfloat16)

    sH = slope.reshape(H).astype(np.float64)
    Dd = np.exp(-sH * TLOC)

    htab_bf = htab.astype(ml_dtypes.bfloat16)
    qdbt_bf = qdbt.astype(ml_dtypes.bfloat16)

    in_maps = []
    for c in range(NCORES):
        b, sidx = c // GRP, c % GRP
        hTc = np.ascontiguousarray(
            hidden[b, sidx * TLOC:(sidx + 1) * TLOC, :].T
        ).astype(ml_dtypes.bfloat16)
        pw = np.zeros((H, GRP), np.float64)
        for j in range(sidx):
            pw[:, j] = Dd ** (sidx - 1 - j)
        pwt = np.broadcast_to(pw.astype(np.float32).reshape(1, H * GRP),
                              (128, H * GRP)).copy()
        in_maps.append(dict(hT=hTc, wq=wqt, wk=wkt, wv=wvt, wg=wgt, wo=wo,
                            htab=htab_bf, qdbt=qdbt_bf, ktab=ktab, pwt=pwt))
    return in_maps


def assemble_out(results):
    out = np.zeros((B, N, HID), np.float32)
    for c in range(NCORES):
        b, sidx = c // GRP, c % GRP
        out[b, sidx * TLOC:(sidx + 1) * TLOC, :] = \
            results[c]["out_t"].astype(np.float32).T
    return out


def kernel(**inputs):
    in_maps = make_in_maps(inputs)
    nc = _get_program()
    res = run_bass_kernel_spmd(nc, in_maps, core_ids=list(range(NCORES)))
    return assemble_out(res.results)
